# revision 28
# baseline (speedup 1.0000x reference)
"""Trainium2 Bass kernel for DiceFromLabelsLoss (histogram binning).

Strategy: data-parallel over the flattened voxel dim across 8 cores (each
core gets half of one sample). Per core, 27 class-masks (c_pred 1..9,
intersection via w = 11*yp + yt == 12c, c_true 1..9) are built on the DVE
as plain bf16 is_equal passes (4x mode), and reduced by the TensorEngine:
a ones[128,1] stationary matmul column-sums each mask slab into a
per-class PSUM slice, PSUM-accumulating across slabs and chunks. PSUM
slices live at partitions {0,32,64,96} (tile_position col-groups) x 7
bank slots. A final DVE reduce drains PSUM to a [128, 8] tile, DMA'd out;
the host does the tiny final dice reduction.

accum_out (TensorScalarPtrReduce) is deliberately NOT used: measured on
HW it is ~10x slower than a plain tensor_scalar pass.
"""

import numpy as np

NUM_CLASSES = 10
N_CORES = 8
SHAPE = (4, 1, 160, 160, 160)
N_SAMPLES = 4
V_TOTAL = 4 * 160 * 160 * 160          # 16_384_000
V_CORE = V_TOTAL // N_CORES            # 2_048_000
P = 128
F = V_CORE // P                        # 16000
NCHUNK = 2
FC = F // NCHUNK                       # 8000
MM_N = 500                             # matmul slab width (psum slot pitch 512)
CHUNK_PLAN = [(0, 2000), (2000, 6000), (8000, 8000)]  # (offset, size) in F cols
N_CLS = 27
N_FOLD = 9                             # masks pre-folded on DVE before the PE

ACT_CLS = {19, 20, 22, 23, 26}

_CACHE = {}


def _build_bass(repeat=1, variant="full"):
    import concourse.bacc as bacc
    import concourse.mybir as mybir
    import concourse.tile as tile

    nc = bacc.Bacc(None, target_bir_lowering=False)
    yp_d = nc.dram_tensor("yp", [P, F], mybir.dt.int32, kind="ExternalInput")
    yt_d = nc.dram_tensor("yt", [P, F], mybir.dt.int32, kind="ExternalInput")
    out_d = nc.dram_tensor("out", [P, 8], mybir.dt.float32, kind="ExternalOutput")

    eq = mybir.AluOpType.is_equal
    bf16 = mybir.dt.bfloat16
    f32 = mybir.dt.float32
    n_slabs = FC // MM_N

    with tile.TileContext(nc) as tc:
        with (
            tc.tile_pool(name="io", bufs=2) as io_pool,
            tc.tile_pool(name="work", bufs=2) as work_pool,
            tc.tile_pool(name="mask", bufs=3) as mask_pool,
            tc.tile_pool(name="fold", bufs=2) as fold_pool,
            tc.tile_pool(name="act", bufs=2) as act_pool,
            tc.tile_pool(name="act1", bufs=1) as act1_pool,
            tc.tile_pool(name="acc", bufs=1) as acc_pool,
            tc.tile_pool(name="psum", bufs=1, space="PSUM") as psum_pool,
        ):
            ones = acc_pool.tile([P, 1], bf16)
            nc.gpsimd.memset(ones[:], 1.0)
            sqb = acc_pool.tile([P, N_CLS], f32)
            for i2 in range(N_CLS):
                if i2 in ACT_CLS:
                    cv = (i2 - 18 + 1) if i2 >= 18 else 0
                    nc.gpsimd.memset(sqb[:, i2:i2 + 1], -float(cv))
            acc = acc_pool.tile([P, 8], f32)
            nc.gpsimd.memset(acc[:], 0.0)
            # one psum tile spanning 7 banks; class i uses
            # [32*(i%4) : 32*(i%4)+1, 512*(i//4) : 512*(i//4)+MM_N]
            psum = psum_pool.tile([P, 7 * 512], f32)

            chunk_plan = CHUNK_PLAN
            n_chunks = len(chunk_plan)
            for k0 in range(n_chunks * repeat):
                k = k0 % n_chunks
                off, FCk = chunk_plan[k]
                ypc = io_pool.tile([P, FC], bf16, tag="ypc")
                ytc = io_pool.tile([P, FC], bf16, tag="ytc")
                ypc = ypc[:, :FCk]
                ytc = ytc[:, :FCk]
                # SWDGE dma casts int32 -> bf16 during the transfer
                nc.gpsimd.dma_start(ypc[:], yp_d[:, off:off + FCk])
                nc.gpsimd.dma_start(ytc[:], yt_d[:, off:off + FCk])

                # w = 11*yp + yt in [0, 109]; intersection_c == count(w == 12c)
                w = work_pool.tile([P, FC], bf16, tag="w")
                w = w[:, :FCk]
                nc.vector.scalar_tensor_tensor(
                    out=w[:], in0=ypc[:], scalar=11.0, in1=ytc[:],
                    op0=mybir.AluOpType.mult, op1=mybir.AluOpType.add,
                )

                jobs = (
                    [(ypc, float(c)) for c in range(1, 10)]
                    + [(w, float(12 * c)) for c in range(1, 10)]
                    + [(ytc, float(c)) for c in range(1, 10)]
                )
                for i, (src, cval) in enumerate(jobs):
                    if variant == "pe_only":
                        if k0 == 0 and i == 0:
                            pe_mask = acc_pool.tile([P, FC], bf16)
                            _CACHE["pe_mask"] = pe_mask
                            nc.vector.tensor_scalar(
                                out=_CACHE["pe_mask"][:], in0=src[:],
                                scalar1=cval, scalar2=0.0,
                                op0=eq, op1=mybir.AluOpType.add,
                            )
                        mask = _CACHE["pe_mask"][:, :FCk]
                    elif i in ACT_CLS and variant == "full":
                        # ScalarE 2-pass mask: relu(1 - (v - c)^2)
                        at = act1_pool.tile([P, FC], bf16, tag="actt")
                        at = at[:, :FCk]
                        nc.scalar.activation(
                            out=at[:], in_=src[:],
                            func=mybir.ActivationFunctionType.Square,
                            bias=sqb[:, i:i + 1],
                        )
                        mask = act_pool.tile([P, FC], bf16, tag="actm")
                        mask = mask[:, :FCk]
                        nc.scalar.activation(
                            out=mask[:], in_=at[:],
                            func=mybir.ActivationFunctionType.Relu,
                            bias=1.0, scale=-1.0,
                        )
                    else:
                        mask = mask_pool.tile([P, FC], bf16, tag="mask")
                        mask = mask[:, :FCk]
                        nc.vector.tensor_scalar(
                            out=mask[:], in0=src[:], scalar1=cval, scalar2=0.0,
                            op0=eq, op1=mybir.AluOpType.add,
                        )
                    if variant == "dve_only":
                        if k0 == NCHUNK * repeat - 1 and i == N_CLS - 1:
                            nc.tensor.matmul(
                                psum[0:1, 0:MM_N], ones[:], mask[:, 0:MM_N],
                                start=True, stop=True, tile_position=(0, 0),
                            )
                        continue
                    if (i % 4 == 1 or i in (3, 7)) and i not in ACT_CLS:
                        # DVE folds mask halves ({0,1,2} values) to halve
                        # the PE stream for this class
                        fm = fold_pool.tile([P, FC // 2], bf16, tag="fmask")
                        fm = fm[:, :FCk // 2]
                        nc.vector.tensor_tensor(
                            out=fm[:], in0=mask[:, :FCk // 2],
                            in1=mask[:, FCk // 2:], op=mybir.AluOpType.add,
                        )
                        feed, fw = fm, FCk // 2
                    else:
                        feed, fw = mask, FCk
                    grp, slot = i % 4, i // 4
                    prow = 32 * grp
                    for s in range(fw // MM_N):
                        nc.tensor.matmul(
                            psum[prow:prow + 1,
                                 512 * slot:512 * slot + MM_N],
                            ones[:],
                            feed[:, s * MM_N:(s + 1) * MM_N],
                            start=(k == 0 and s == 0),
                            stop=(k == n_chunks - 1 and s == fw // MM_N - 1),
                            tile_position=(0, prow),
                        )

            # drain PSUM: per partition-group, reduce [1, 7, MM_N] -> [1, 7]
            for grp in range(4):
                prow = 32 * grp
                view = psum[prow:prow + 1, :].rearrange(
                    "p (b n) -> p b n", n=512
                )[:, :, 0:MM_N]
                nc.vector.tensor_reduce(
                    out=acc[prow:prow + 1, 0:7], in_=view,
                    axis=mybir.AxisListType.X, op=mybir.AluOpType.add,
                )
            nc.sync.dma_start(out_d[:], acc[:])
    nc.finalize()
    return nc


def _get_built():
    if "nc" not in _CACHE:
        _CACHE["nc"] = _build_bass()
    return _CACHE["nc"]


def _decode_counts(raw):
    """raw: [P, 8] per-core output -> (cp[9], ct[9], inter[9])."""
    vals = np.zeros(N_CLS, np.float64)
    for i in range(N_CLS):
        vals[i] = raw[32 * (i % 4), i // 4]
    return vals[0:9], vals[18:27], vals[9:18]


def _host_finish(per_core_raw):
    cp = np.zeros((N_SAMPLES, 9), np.float64)
    ct = np.zeros((N_SAMPLES, 9), np.float64)
    it = np.zeros((N_SAMPLES, 9), np.float64)
    cores_per_sample = N_CORES // N_SAMPLES
    for core, raw in enumerate(per_core_raw):
        s = core // cores_per_sample
        a, b, c = _decode_counts(raw)
        cp[s] += a
        ct[s] += b
        it[s] += c
    denom = cp + ct
    nonzero = denom > 0
    denom_safe = np.where(nonzero, denom, 1.0)
    dice_terms = np.where(nonzero, 2.0 * it / denom_safe, 0.0)
    weight = ct / ct.sum(-1, keepdims=True) / N_SAMPLES
    loss = 1.0 - np.sum(np.where(nonzero, weight, 0.0) * dice_terms)
    return np.array(loss, dtype=np.float32)


def _make_in_maps(y_pred, y_true):
    yp = np.ascontiguousarray(np.asarray(y_pred).reshape(-1)).astype(
        np.int32, copy=False
    )
    yt = np.ascontiguousarray(np.asarray(y_true).reshape(-1)).astype(
        np.int32, copy=False
    )
    in_maps = []
    for core in range(N_CORES):
        sl = slice(core * V_CORE, (core + 1) * V_CORE)
        in_maps.append({
            "yp": yp[sl].reshape(P, F),
            "yt": yt[sl].reshape(P, F),
        })
    return in_maps


def _run(in_maps, **kw):
    from concourse.bass_utils import run_bass_kernel_spmd

    nc = _get_built()
    res = run_bass_kernel_spmd(nc, in_maps, core_ids=list(range(N_CORES)), **kw)
    per_core = [r["out"] for r in res.results]
    return per_core, res


def kernel(y_pred, y_true):
    per_core, _ = _run(_make_in_maps(y_pred, y_true))
    return _host_finish(per_core)


if __name__ == "__main__":
    rng = np.random.default_rng(0)
    a = rng.integers(0, 10, SHAPE, dtype=np.int32)
    b = rng.integers(0, 10, SHAPE, dtype=np.int32)
    print(kernel(a, b))
